# revision 6
# baseline (speedup 1.0000x reference)
"""DigitCaps dynamic-routing kernel for 8x TRN2 NeuronCores (v3, q-pipelined).

Per core (64 batch), routing never materializes u_hat:
  s0   = 0.1 * x @ W                       (PE, K=(i,d) contraction)
  v    = squash(s) = P(z^2) - 0.5|z| - 1   (polynomial; one act table)
  per q-chunk of 128 input capsules, fully pipelined:
    A(q,p)   = W x_D v          (PE, K=32=(o2,D) per o-pair, out (d,i128))
    prod     = A * x            (ACT evict + DVE mult, some pairs fused on DVE)
    bvalT(q) = sum_d prod       (PE: 8 accumulating identity-matmuls = transpose+tree)
    c(q)     = exp(bvalT)       (ACT, from PSUM)
    y(q)     = c * x/Z          (DVE/Pool split)
    s       += y(q,d,o) @ W     (PE, y as lhsT, per-o N=16 rhs, b-layout PSUM out)
  v_new = squash(s); b2 = b1 + UV(v1) = UV(v0+v1) by linearity.
"""

import os
import numpy as np
import ml_dtypes

import concourse.bass as bass
import concourse.bacc as bacc
import concourse.mybir as mybir
from concourse.tile import TileContext
from concourse.bass_utils import run_bass_kernel_spmd

bf16 = ml_dtypes.bfloat16
F32 = mybir.dt.float32
BF = mybir.dt.bfloat16
AF = mybir.ActivationFunctionType
ALU = mybir.AluOpType
AX = mybir.AxisListType

B, O, I, D, d = 512, 10, 1152, 16, 8
BL = 64          # batch per core
NPAIR = 5        # o-pairs
NQ = 9           # i chunks of 128
NT = 72          # (q, d) tiles

_B = lambda k, dflt: int(os.environ.get(k, dflt))
def _sl(t):
    import concourse.bass as _bass
    return t if isinstance(t, _bass.AP) else t[:]

# squash: v = P(z^2) - 0.5|z| - 1, P(w) ~= cos(sqrt w) - 0.5 sqrt(w) erf(sqrt(w/2))
# fit over |z| <= 1.2, deg 3, max err ~4e-5; -1 folded into c0
_PC = [-2.1933e-05, -0.8983608275706338, 0.10590715814988215,
       -0.008555334079961922]


def _squash(nc, pool, ps_ap, shape, tag, scale):
    """v = squash(scale*ps) via poly; ps_ap is a PSUM AP; returns f32 tile.
    Intermediates in bf16 for DVE 2x/4x modes (|err| ~1e-3 << gate)."""
    IDT = BF if _B("SQBF", 1) else F32
    a = pool.tile(shape, IDT, tag=f"{tag}_a")
    w = pool.tile(shape, IDT, tag=f"{tag}_w")
    r = pool.tile(shape, IDT, tag=f"{tag}_r")
    v = pool.tile(shape, F32, tag=f"{tag}_v")
    nc.scalar.activation(a[:], ps_ap, AF.Abs, scale=scale)
    nc.scalar.activation(w[:], ps_ap, AF.Square, scale=scale)
    with nc.allow_low_precision(reason="poly intermediates; gate margin 6x"):
        nc.vector.tensor_scalar(r[:], w[:], _PC[3], _PC[2], ALU.mult, ALU.add)
        for k in (1, 0):
            nc.vector.tensor_tensor(r[:], r[:], w[:], ALU.mult)
            nc.vector.tensor_scalar(r[:], r[:], _PC[k], None, ALU.add)
        nc.vector.scalar_tensor_tensor(v[:], a[:], -0.5, r[:], ALU.mult, ALU.add)
    return v


def _body(nc, tc, x3_d, xdi_d, w2_d, w1s_d, idb_d, msk_d, out_d):
    NDQ = _B("NDQ", 5)        # of 45 (q,p) chunks: how many fuse evict+mult on DVE
    if _B("DSP", 0) == 0:
        direct_set = {round(i * 45 / NDQ) + 7 for i in range(NDQ)} if NDQ else set()
    else:
        direct_set = {q * 5 + 3 for q in range(0, 2 * NDQ, 2)} if NDQ else set()
    YPD = _B("YPD", 2)        # y d-slices on Pool (of 8); tail q runs all-DVE
    with (
        tc.tile_pool(name="const", bufs=1) as cpool,
        tc.tile_pool(name="work", bufs=1) as wpool,
        tc.tile_pool(name="small", bufs=3) as spool,
        tc.tile_pool(name="asb", bufs=_B("ASB", 4)) as apool,
        tc.tile_pool(name="ypool", bufs=_B("YB", 3)) as ypool,
        tc.tile_pool(name="psA2", bufs=_B("PSA2", 2), space="PSUM") as psA2_pool,
        tc.tile_pool(name="psBR", bufs=_B("PSBR", 1), space="PSUM") as psBR_pool,
        tc.tile_pool(name="psS", bufs=1, space="PSUM") as psS_pool,
    ):
        # ---- resident loads: x3 whole, w1s in 3 chunk-tiles (fine-grained
        # deps let s0 chase the DMA), then per-q xdi/w2 ----
        x3 = cpool.tile([128, NQ, d, BL], BF)
        w1sc = [cpool.tile([128, 24, 160], BF, name=f"w1s{j}") for j in range(3)]
        xdi = cpool.tile([128, NQ, d, 128], BF)
        w2 = cpool.tile([32, NQ, NPAIR, d * 128], BF)
        idb = cpool.tile([128, 128], BF)
        msk = cpool.tile([128, 2], F32)
        nc.sync.dma_start(idb[:], idb_d.ap())
        nc.sync.dma_start(msk[:], msk_d.ap())
        nc.sync.dma_start(x3[:], x3_d.ap())
        for j in range(3):
            nc.sync.dma_start(w1sc[j][:], w1s_d.ap()[:, 24 * j:24 * (j + 1)])
        for q in range(NQ):
            nc.sync.dma_start(xdi[:, q], xdi_d.ap()[:, q])
            nc.sync.dma_start(w2[:, q], w2_d.ap()[:, q])

        # ---- PE p-state warmup on idb during the input-DMA window ----
        NWARM = _B("NWARM", 0)
        if NWARM:
            warm = psA2_pool.tile([128, 128], F32, tag="warm", bufs=1, name="warm")
            for i in range(NWARM):
                nc.tensor.matmul(warm[:], idb[:], idb[:], start=True, stop=True)
        # ---- phase s0 ----
        ps0 = psS_pool.tile([BL, 160], F32, tag="psS", name="ps0")
        for t in range(NT):
            q, d_ = divmod(t, d)
            nc.tensor.matmul(ps0[:], x3[:, q, d_], w1sc[t // 24][:, t % 24],
                             start=(t == 0), stop=(t == NT - 1))
        v0b = _squash(nc, wpool, ps0[:], [BL, 160], "sq0", 0.1)
        vsum_b = wpool.tile([BL, 160], F32, tag="vsum")
        nc.vector.tensor_copy(vsum_b[:], v0b[:])

        vnew = None
        for it in range(2):
            vin_b = v0b if it == 0 else vsum_b
            # vT via DVE 32x32 stream transposes; blk = masked per-pair lhsT
            vT = wpool.tile([32, NPAIR, BL], F32, tag="vT", bufs=2, name=f"vT{it}")
            blk = wpool.tile([32, NPAIR, 128], BF, tag="blk", bufs=2, name=f"blk{it}")
            for p in range(NPAIR):
                nc.vector.transpose(vT[:, p, 0:32], vin_b[0:32, 32 * p:32 * p + 32])
                nc.vector.transpose(vT[:, p, 32:64], vin_b[32:64, 32 * p:32 * p + 32])
                nc.vector.tensor_scalar(blk[:, p, 0:64], vT[:, p], msk[0:32, 0:1], None, ALU.mult)
                nc.vector.tensor_scalar(blk[:, p, 64:128], vT[:, p], msk[0:32, 1:2], None, ALU.mult)

            psSa = psS_pool.tile([BL, 160], F32, tag="psS", name=f"psS{it}")
            prev_y = None
            pending_smax = None
            PMQ = _B("PMQ", -1)     # pair index handled by Pool mult (-1: none)
            for q in range(NQ):
                # ---- UV(q): A-matmuls, evict+mult, tree-transposes ----
                if _B("BRSPLIT", 0):
                    pstA = psBR_pool.tile([128, 512], F32, tag="brA", bufs=_B("BRA", 2),
                                          name=f"pstA{it}_{q}")
                    pstB = psBR_pool.tile([128, 128], F32, tag="brB", bufs=_B("BRB", 1),
                                          name=f"pstB{it}_{q}")
                else:
                    pst5 = psBR_pool.tile([128, 640], F32, tag="br", bufs=1,
                                          name=f"pstb{it}_{q}")
                    pstA = pst5[:, 0:512].rearrange("p (x) -> p x")
                    pstB = pst5[:, 512:640].rearrange("p (x) -> p x")
                asbs = []
                def _uv_chunk(p):
                    psA = psA2_pool.tile([128, 1024], F32, tag="psA",
                                         name=f"psA{it}_{q}_{p}")
                    nc.tensor.matmul(psA[:, 0:512], blk[:, p], w2[:, q, p, 0:512],
                                     start=True, stop=True)
                    nc.tensor.matmul(psA[:, 512:1024], blk[:, p], w2[:, q, p, 512:1024],
                                     start=True, stop=True)
                    A_sb = apool.tile([128, d, 128], BF, tag="A_sb")
                    xstripe = xdi[:, q]
                    if (q * NPAIR + p) in direct_set:
                        nc.vector.tensor_tensor(
                            A_sb[:], psA[:].rearrange("p (a b) -> p a b", a=d),
                            xstripe, ALU.mult)
                    else:
                        nc.scalar.copy(A_sb[:].rearrange("p a b -> p (a b)"), psA[:])
                        eng = nc.gpsimd if p == PMQ else nc.vector
                        eng.tensor_tensor(A_sb[:], A_sb[:], xstripe, ALU.mult)
                    asbs.append(A_sb)
                def _tree(p):
                    tgt = _sl(pstB) if p == 4 else _sl(pstA)[:, 128 * p:128 * (p + 1)]
                    for dd in range(d):
                        nc.tensor.matmul(tgt, asbs[p][:, dd], idb[:],
                                         start=(dd == 0 and p in (0, 4)),
                                         stop=(dd == d - 1))
                for p in range(NPAIR):
                    _uv_chunk(p)
                if _B("EXPD", 0) and pending_smax is not None:
                    pending_smax()      # exp(q-1) lands after evicts(q) on ACT
                    pending_smax = None
                if _B("SPOS", 0) == 1 and prev_y is not None:
                    _emit_S(nc, psSa, prev_y, w1sc, q - 1)
                    prev_y = None
                for p in range(NPAIR):
                    _tree(p)
                if _B("SORD", 1) and prev_y is not None:
                    _emit_S(nc, psSa, prev_y, w1sc, q - 1)
                    prev_y = None
                # ---- softmax(q) + y(q) ----
                def _smax(q=q, pstA=pstA, pstB=pstB, pst5=(None if _B("BRSPLIT", 0) else pst5)):
                    nonlocal prev_y
                    cTq = spool.tile([128, O, BL], BF, tag="cT")
                    if _B("BRSPLIT", 0):
                        nc.scalar.activation(cTq[:, 0:8],
                                             _sl(pstA).rearrange("p (o b) -> p o b", o=8), AF.Exp)
                        nc.scalar.activation(cTq[:, 8:10],
                                             _sl(pstB).rearrange("p (o b) -> p o b", o=2), AF.Exp)
                    else:
                        nc.scalar.activation(cTq[:],
                                             pst5[:].rearrange("p (o b) -> p o b", o=O), AF.Exp)
                    Z = spool.tile([128, BL], F32, tag="Z")
                    nc.vector.tensor_reduce(Z[:], cTq[:].rearrange("p o b -> p b o"),
                                            AX.X, ALU.add)
                    rec = spool.tile([128, BL], BF, tag="rec")
                    with nc.allow_low_precision(reason="1/Z feeds bf16 xs anyway"):
                        nc.vector.reciprocal(rec[:], Z[:])
                    xsq = spool.tile([128, d, BL], BF, tag="xsq")
                    nc.vector.tensor_tensor(xsq[:], x3[:, q],
                                            rec[:].unsqueeze(1).broadcast_to((128, d, BL)),
                                            ALU.mult)
                    yq = ypool.tile([128, d, O, BL], BF, tag="y", name=f"y{it}_{q}")
                    ypd = YPD if q < NQ - 1 else 0
                    dsp = d - ypd
                    if q == NQ - 1:
                        # tail q: split so S can start on early d-slices sooner
                        for dh in range(0, d, 2):
                            nc.vector.tensor_tensor(
                                yq[:, dh:dh + 2],
                                cTq[:].unsqueeze(1).broadcast_to((128, 2, O, BL)),
                                xsq[:, dh:dh + 2].unsqueeze(2).broadcast_to((128, 2, O, BL)),
                                ALU.mult)
                    elif dsp:
                        nc.vector.tensor_tensor(
                            yq[:, 0:dsp],
                            cTq[:].unsqueeze(1).broadcast_to((128, dsp, O, BL)),
                            xsq[:, 0:dsp].unsqueeze(2).broadcast_to((128, dsp, O, BL)),
                            ALU.mult)
                    for ddp in range(dsp, d):
                        nc.gpsimd.tensor_tensor(
                            yq[:, ddp:ddp + 1],
                            cTq[:].unsqueeze(1).broadcast_to((128, 1, O, BL)),
                            xsq[:, ddp:ddp + 1].unsqueeze(2).broadcast_to((128, 1, O, BL)),
                            ALU.mult)
                    if prev_y is not None:
                        _emit_S(nc, psSa, prev_y, w1sc, q - 1)
                    prev_y = yq
                if _B("EXPD", 0):
                    pending_smax = _smax
                else:
                    _smax()
            if pending_smax is not None:
                pending_smax()
            _emit_S(nc, psSa, prev_y, w1sc, NQ - 1)

            vnew = _squash(nc, wpool, psSa[:], [BL, 160], "sqi", 1.0)
            if it == 0:
                nc.vector.tensor_tensor(vsum_b[:], vsum_b[:], vnew[:], ALU.add)

        # ---- output: vnew [64, 160] == out[b, (o, D)] ----
        nc.sync.dma_start(out_d.ap().rearrange("b o D -> b (o D)"), vnew[:])


def _emit_S(nc, psSa, yq, w1sc, q):
    for dd in range(d):
        t = q * d + dd
        for o in range(O):
            nc.tensor.matmul(psSa[:, 16 * o:16 * o + 16], yq[:, dd, o],
                             w1sc[t // 24][:, t % 24, 16 * o:16 * o + 16],
                             start=(t == 0 and o == 0),
                             stop=(t == NT - 1 and o == O - 1))


def build_program():
    nc = bacc.Bacc("TRN2", debug=False, target_bir_lowering=False)
    x3_d = nc.dram_tensor("x3", [128, NQ, d, BL], BF, kind="ExternalInput")
    xdi_d = nc.dram_tensor("xdi", [128, NQ, d, 128], BF, kind="ExternalInput")
    w2_d = nc.dram_tensor("w2", [32, NQ, NPAIR, d * 128], BF, kind="ExternalInput")
    w1s_d = nc.dram_tensor("w1s", [128, NT, 160], BF, kind="ExternalInput")
    idb_d = nc.dram_tensor("idb", [128, 128], BF, kind="ExternalInput")
    msk_d = nc.dram_tensor("msk", [128, 2], F32, kind="ExternalInput")
    out_d = nc.dram_tensor("out", [BL, O, D], F32, kind="ExternalOutput")
    with TileContext(nc) as tc:
        _body(nc, tc, x3_d, xdi_d, w2_d, w1s_d, idb_d, msk_d, out_d)
    nc.compile()
    return nc


def host_prep_w(W):
    """W: [1,10,1152,16,8] fp32 -> (w2, w1s, idb, msk) arrays."""
    Wb = W[0].astype(bf16)
    # w2[(o2,D), q, p, (d, i128)]
    w2 = np.ascontiguousarray(
        Wb.reshape(5, 2, NQ, 128, D, d).transpose(1, 4, 2, 0, 5, 3)
    ).reshape(32, NQ, NPAIR, d * 128)
    w1s = np.ascontiguousarray(
        Wb.reshape(5, 2, NQ, 128, D, d).transpose(3, 2, 5, 0, 1, 4)).reshape(128, NT, 160)
    idb = np.eye(128, dtype=bf16)
    msk = np.zeros((128, 2), np.float32)
    msk[:, 0] = np.tile(np.r_[np.ones(16), np.zeros(16)], 4)
    msk[:, 1] = 1.0 - msk[:, 0]
    return w2, w1s, idb, msk


def host_prep_x(xc):
    """xc: [64, 1152, 8] fp32 -> (x3, xdi)."""
    xb = xc.astype(bf16)
    x3 = np.ascontiguousarray(xb.reshape(BL, NQ, 128, d).transpose(2, 1, 3, 0))
    xd = np.ascontiguousarray(xb.transpose(0, 2, 1))        # [64, 8, 1152]
    xdi = np.concatenate([xd, xd], axis=0)                  # [128, 8, 1152]
    xdi = np.ascontiguousarray(
        xdi.reshape(128, d, NQ, 128).transpose(0, 2, 1, 3))  # [128, q, d, 128]
    return x3, xdi


_NC_CACHE = {}


def _get_nc():
    if "nc" not in _NC_CACHE:
        _NC_CACHE["nc"] = build_program()
    return _NC_CACHE["nc"]


def kernel(x, W):
    x = np.asarray(x, dtype=np.float32)
    W = np.asarray(W, dtype=np.float32)
    w2, w1s, idb, msk = host_prep_w(W)
    in_maps = []
    for core in range(8):
        x3, xdi = host_prep_x(x[core * BL:(core + 1) * BL])
        in_maps.append({"x3": x3, "xdi": xdi, "w2": w2, "w1s": w1s,
                        "idb": idb, "msk": msk})
    nc = _get_nc()
    res = run_bass_kernel_spmd(nc, in_maps, list(range(8)))
    out = np.concatenate([res.results[i]["out"] for i in range(8)], axis=0)
    return out.astype(np.float32)


# revision 7
# speedup vs baseline: 1.0058x; 1.0058x over previous
"""DigitCaps dynamic-routing kernel for 8x TRN2 NeuronCores (v3, q-pipelined).

Per core (64 batch), routing never materializes u_hat:
  s0   = 0.1 * x @ W                       (PE, K=(i,d) contraction)
  v    = squash(s) = P(z^2) - 0.5|z| - 1   (polynomial; one act table)
  per q-chunk of 128 input capsules, fully pipelined:
    A(q,p)   = W x_D v          (PE, K=32=(o2,D) per o-pair, out (d,i128))
    prod     = A * x            (ACT evict + DVE mult, some pairs fused on DVE)
    bvalT(q) = sum_d prod       (PE: 8 accumulating identity-matmuls = transpose+tree)
    c(q)     = exp(bvalT)       (ACT, from PSUM)
    y(q)     = c * x/Z          (DVE/Pool split)
    s       += y(q,d,o) @ W     (PE, y as lhsT, per-o N=16 rhs, b-layout PSUM out)
  v_new = squash(s); b2 = b1 + UV(v1) = UV(v0+v1) by linearity.
"""

import os
import numpy as np
import ml_dtypes

import concourse.bass as bass
import concourse.bacc as bacc
import concourse.mybir as mybir
from concourse.tile import TileContext
from concourse.bass_utils import run_bass_kernel_spmd

bf16 = ml_dtypes.bfloat16
F32 = mybir.dt.float32
BF = mybir.dt.bfloat16
AF = mybir.ActivationFunctionType
ALU = mybir.AluOpType
AX = mybir.AxisListType

B, O, I, D, d = 512, 10, 1152, 16, 8
BL = 64          # batch per core
NPAIR = 5        # o-pairs
NQ = 9           # i chunks of 128
NT = 72          # (q, d) tiles

_B = lambda k, dflt: int(os.environ.get(k, dflt))
def _sl(t):
    import concourse.bass as _bass
    return t if isinstance(t, _bass.AP) else t[:]

# squash: v = P(z^2) - 0.5|z| - 1, P(w) ~= cos(sqrt w) - 0.5 sqrt(w) erf(sqrt(w/2))
# fit over |z| <= 1.2, deg 3, max err ~4e-5; -1 folded into c0
_PC = [-2.1933e-05, -0.8983608275706338, 0.10590715814988215,
       -0.008555334079961922]


def _squash(nc, pool, ps_ap, shape, tag, scale, out_dt=F32):
    """v = squash(scale*ps) via poly; ps_ap is a PSUM AP; returns f32 tile.
    Intermediates in bf16 for DVE 2x/4x modes (|err| ~1e-3 << gate)."""
    IDT = BF if _B("SQBF", 1) else F32
    a = pool.tile(shape, IDT, tag=f"{tag}_a")
    w = pool.tile(shape, IDT, tag=f"{tag}_w")
    r = pool.tile(shape, IDT, tag=f"{tag}_r")
    v = pool.tile(shape, out_dt, tag=f"{tag}_v")
    nc.scalar.activation(a[:], ps_ap, AF.Abs, scale=scale)
    nc.scalar.activation(w[:], ps_ap, AF.Square, scale=scale)
    with nc.allow_low_precision(reason="poly intermediates; gate margin 6x"):
        nc.vector.tensor_scalar(r[:], w[:], _PC[3], _PC[2], ALU.mult, ALU.add)
        for k in (1, 0):
            nc.vector.tensor_tensor(r[:], r[:], w[:], ALU.mult)
            nc.vector.tensor_scalar(r[:], r[:], _PC[k], None, ALU.add)
        nc.vector.scalar_tensor_tensor(v[:], a[:], -0.5, r[:], ALU.mult, ALU.add)
    return v


def _body(nc, tc, x3_d, xdi_d, w2_d, w1s_d, idb_d, msk_d, out_d):
    NDQ = _B("NDQ", 5)        # of 45 (q,p) chunks: how many fuse evict+mult on DVE
    if _B("DSP", 0) == 0:
        direct_set = {round(i * 45 / NDQ) + 7 for i in range(NDQ)} if NDQ else set()
    else:
        direct_set = {q * 5 + 3 for q in range(0, 2 * NDQ, 2)} if NDQ else set()
    YPD = _B("YPD", 2)        # y d-slices on Pool (of 8); tail q runs all-DVE
    with (
        tc.tile_pool(name="const", bufs=1) as cpool,
        tc.tile_pool(name="work", bufs=1) as wpool,
        tc.tile_pool(name="small", bufs=3) as spool,
        tc.tile_pool(name="asb", bufs=_B("ASB", 4)) as apool,
        tc.tile_pool(name="ypool", bufs=_B("YB", 3)) as ypool,
        tc.tile_pool(name="psA2", bufs=_B("PSA2", 2), space="PSUM") as psA2_pool,
        tc.tile_pool(name="psBR", bufs=_B("PSBR", 1), space="PSUM") as psBR_pool,
        tc.tile_pool(name="psS", bufs=1, space="PSUM") as psS_pool,
    ):
        # ---- resident loads: x3 whole, w1s in 3 chunk-tiles (fine-grained
        # deps let s0 chase the DMA), then per-q xdi/w2 ----
        x3 = cpool.tile([128, NQ, d, BL], BF)
        w1sc = [cpool.tile([128, 24, 160], BF, name=f"w1s{j}") for j in range(3)]
        xdi = cpool.tile([128, NQ, d, 128], BF)
        w2 = cpool.tile([32, NQ, NPAIR, d * 128], BF)
        idb = cpool.tile([128, 128], BF)
        msk = cpool.tile([128, 2], F32)
        nc.sync.dma_start(idb[:], idb_d.ap())
        nc.sync.dma_start(msk[:], msk_d.ap())
        nc.sync.dma_start(x3[:], x3_d.ap())
        for j in range(3):
            nc.sync.dma_start(w1sc[j][:], w1s_d.ap()[:, 24 * j:24 * (j + 1)])
        for q in range(NQ):
            nc.sync.dma_start(xdi[:, q], xdi_d.ap()[:, q])
            nc.sync.dma_start(w2[:, q], w2_d.ap()[:, q])

        # ---- PE p-state warmup on idb during the input-DMA window ----
        NWARM = _B("NWARM", 0)
        if NWARM:
            warm = psA2_pool.tile([128, 128], F32, tag="warm", bufs=1, name="warm")
            for i in range(NWARM):
                nc.tensor.matmul(warm[:], idb[:], idb[:], start=True, stop=True)
        # ---- phase s0 ----
        ps0 = psS_pool.tile([BL, 160], F32, tag="psS", name="ps0")
        for t in range(NT):
            q, d_ = divmod(t, d)
            nc.tensor.matmul(ps0[:], x3[:, q, d_], w1sc[t // 24][:, t % 24],
                             start=(t == 0), stop=(t == NT - 1))
        VDT = BF if _B("VBF", 1) else F32
        v0b = _squash(nc, wpool, ps0[:], [BL, 160], "sq0", 0.1, VDT)
        vsum_b = wpool.tile([BL, 160], VDT, tag="vsum")
        nc.vector.tensor_copy(vsum_b[:], v0b[:])

        vnew = None
        for it in range(2):
            vin_b = v0b if it == 0 else vsum_b
            # vT via DVE 32x32 stream transposes; blk = masked per-pair lhsT
            vT = wpool.tile([32, NPAIR, BL], VDT, tag="vT", bufs=2, name=f"vT{it}")
            blk = wpool.tile([32, NPAIR, 128], BF, tag="blk", bufs=2, name=f"blk{it}")
            for p in range(NPAIR):
                nc.vector.transpose(vT[:, p, 0:32], vin_b[0:32, 32 * p:32 * p + 32])
                nc.vector.transpose(vT[:, p, 32:64], vin_b[32:64, 32 * p:32 * p + 32])
                nc.vector.tensor_scalar(blk[:, p, 0:64], vT[:, p], msk[0:32, 0:1], None, ALU.mult)
                nc.vector.tensor_scalar(blk[:, p, 64:128], vT[:, p], msk[0:32, 1:2], None, ALU.mult)

            psSa = psS_pool.tile([BL, 160], F32, tag="psS", name=f"psS{it}")
            prev_y = None
            pending_smax = None
            PMQ = _B("PMQ", -1)     # pair index handled by Pool mult (-1: none)
            for q in range(NQ):
                # ---- UV(q): A-matmuls, evict+mult, tree-transposes ----
                if _B("BRSPLIT", 0):
                    pstA = psBR_pool.tile([128, 512], F32, tag="brA", bufs=_B("BRA", 2),
                                          name=f"pstA{it}_{q}")
                    pstB = psBR_pool.tile([128, 128], F32, tag="brB", bufs=_B("BRB", 1),
                                          name=f"pstB{it}_{q}")
                else:
                    pst5 = psBR_pool.tile([128, 640], F32, tag="br", bufs=1,
                                          name=f"pstb{it}_{q}")
                    pstA = pst5[:, 0:512].rearrange("p (x) -> p x")
                    pstB = pst5[:, 512:640].rearrange("p (x) -> p x")
                asbs = []
                def _uv_chunk(p):
                    psA = psA2_pool.tile([128, 1024], F32, tag="psA",
                                         name=f"psA{it}_{q}_{p}")
                    nc.tensor.matmul(psA[:, 0:512], blk[:, p], w2[:, q, p, 0:512],
                                     start=True, stop=True)
                    nc.tensor.matmul(psA[:, 512:1024], blk[:, p], w2[:, q, p, 512:1024],
                                     start=True, stop=True)
                    A_sb = apool.tile([128, d, 128], BF, tag="A_sb")
                    xstripe = xdi[:, q]
                    if (q * NPAIR + p) in direct_set:
                        nc.vector.tensor_tensor(
                            A_sb[:], psA[:].rearrange("p (a b) -> p a b", a=d),
                            xstripe, ALU.mult)
                    else:
                        nc.scalar.copy(A_sb[:].rearrange("p a b -> p (a b)"), psA[:])
                        eng = nc.gpsimd if p == PMQ else nc.vector
                        eng.tensor_tensor(A_sb[:], A_sb[:], xstripe, ALU.mult)
                    asbs.append(A_sb)
                def _tree(p):
                    tgt = _sl(pstB) if p == 4 else _sl(pstA)[:, 128 * p:128 * (p + 1)]
                    for dd in range(d):
                        nc.tensor.matmul(tgt, asbs[p][:, dd], idb[:],
                                         start=(dd == 0 and p in (0, 4)),
                                         stop=(dd == d - 1))
                for p in range(NPAIR):
                    _uv_chunk(p)
                if _B("EXPD", 0) and pending_smax is not None:
                    pending_smax()      # exp(q-1) lands after evicts(q) on ACT
                    pending_smax = None
                if _B("SPOS", 0) == 1 and prev_y is not None:
                    _emit_S(nc, psSa, prev_y, w1sc, q - 1)
                    prev_y = None
                for p in range(NPAIR):
                    _tree(p)
                if _B("SORD", 1) and prev_y is not None:
                    _emit_S(nc, psSa, prev_y, w1sc, q - 1)
                    prev_y = None
                # ---- softmax(q) + y(q) ----
                def _smax(q=q, pstA=pstA, pstB=pstB, pst5=(None if _B("BRSPLIT", 0) else pst5)):
                    nonlocal prev_y
                    cTq = spool.tile([128, O, BL], BF, tag="cT")
                    if _B("BRSPLIT", 0):
                        nc.scalar.activation(cTq[:, 0:8],
                                             _sl(pstA).rearrange("p (o b) -> p o b", o=8), AF.Exp)
                        nc.scalar.activation(cTq[:, 8:10],
                                             _sl(pstB).rearrange("p (o b) -> p o b", o=2), AF.Exp)
                    else:
                        nc.scalar.activation(cTq[:],
                                             pst5[:].rearrange("p (o b) -> p o b", o=O), AF.Exp)
                    Z = spool.tile([128, BL], F32, tag="Z")
                    nc.vector.tensor_reduce(Z[:], cTq[:].rearrange("p o b -> p b o"),
                                            AX.X, ALU.add)
                    rec = spool.tile([128, BL], BF, tag="rec")
                    with nc.allow_low_precision(reason="1/Z feeds bf16 xs anyway"):
                        nc.vector.reciprocal(rec[:], Z[:])
                    xsq = spool.tile([128, d, BL], BF, tag="xsq")
                    nc.vector.tensor_tensor(xsq[:], x3[:, q],
                                            rec[:].unsqueeze(1).broadcast_to((128, d, BL)),
                                            ALU.mult)
                    yq = ypool.tile([128, d, O, BL], BF, tag="y", name=f"y{it}_{q}")
                    ypd = YPD if q < NQ - 1 else 0
                    dsp = d - ypd
                    if q == NQ - 1:
                        # tail q: split so S can start on early d-slices sooner
                        for dh in range(0, d, 2):
                            nc.vector.tensor_tensor(
                                yq[:, dh:dh + 2],
                                cTq[:].unsqueeze(1).broadcast_to((128, 2, O, BL)),
                                xsq[:, dh:dh + 2].unsqueeze(2).broadcast_to((128, 2, O, BL)),
                                ALU.mult)
                    elif dsp:
                        nc.vector.tensor_tensor(
                            yq[:, 0:dsp],
                            cTq[:].unsqueeze(1).broadcast_to((128, dsp, O, BL)),
                            xsq[:, 0:dsp].unsqueeze(2).broadcast_to((128, dsp, O, BL)),
                            ALU.mult)
                    for ddp in range(dsp, d):
                        nc.gpsimd.tensor_tensor(
                            yq[:, ddp:ddp + 1],
                            cTq[:].unsqueeze(1).broadcast_to((128, 1, O, BL)),
                            xsq[:, ddp:ddp + 1].unsqueeze(2).broadcast_to((128, 1, O, BL)),
                            ALU.mult)
                    if prev_y is not None:
                        _emit_S(nc, psSa, prev_y, w1sc, q - 1)
                    prev_y = yq
                if _B("EXPD", 0):
                    pending_smax = _smax
                else:
                    _smax()
            if pending_smax is not None:
                pending_smax()
            _emit_S(nc, psSa, prev_y, w1sc, NQ - 1)

            vnew = _squash(nc, wpool, psSa[:], [BL, 160], "sqi", 1.0,
                           VDT if it == 0 else F32)
            if it == 0:
                nc.vector.tensor_tensor(vsum_b[:], vsum_b[:], vnew[:], ALU.add)

        # ---- output: vnew [64, 160] == out[b, (o, D)] ----
        nc.sync.dma_start(out_d.ap().rearrange("b o D -> b (o D)"), vnew[:])


def _emit_S(nc, psSa, yq, w1sc, q):
    for dd in range(d):
        t = q * d + dd
        for o in range(O):
            nc.tensor.matmul(psSa[:, 16 * o:16 * o + 16], yq[:, dd, o],
                             w1sc[t // 24][:, t % 24, 16 * o:16 * o + 16],
                             start=(t == 0 and o == 0),
                             stop=(t == NT - 1 and o == O - 1))


def build_program():
    nc = bacc.Bacc("TRN2", debug=False, target_bir_lowering=False)
    x3_d = nc.dram_tensor("x3", [128, NQ, d, BL], BF, kind="ExternalInput")
    xdi_d = nc.dram_tensor("xdi", [128, NQ, d, 128], BF, kind="ExternalInput")
    w2_d = nc.dram_tensor("w2", [32, NQ, NPAIR, d * 128], BF, kind="ExternalInput")
    w1s_d = nc.dram_tensor("w1s", [128, NT, 160], BF, kind="ExternalInput")
    idb_d = nc.dram_tensor("idb", [128, 128], BF, kind="ExternalInput")
    msk_d = nc.dram_tensor("msk", [128, 2], F32, kind="ExternalInput")
    out_d = nc.dram_tensor("out", [BL, O, D], F32, kind="ExternalOutput")
    with TileContext(nc) as tc:
        _body(nc, tc, x3_d, xdi_d, w2_d, w1s_d, idb_d, msk_d, out_d)
    nc.compile()
    return nc


def host_prep_w(W):
    """W: [1,10,1152,16,8] fp32 -> (w2, w1s, idb, msk) arrays."""
    Wb = W[0].astype(bf16)
    # w2[(o2,D), q, p, (d, i128)]
    w2 = np.ascontiguousarray(
        Wb.reshape(5, 2, NQ, 128, D, d).transpose(1, 4, 2, 0, 5, 3)
    ).reshape(32, NQ, NPAIR, d * 128)
    w1s = np.ascontiguousarray(
        Wb.reshape(5, 2, NQ, 128, D, d).transpose(3, 2, 5, 0, 1, 4)).reshape(128, NT, 160)
    idb = np.eye(128, dtype=bf16)
    msk = np.zeros((128, 2), np.float32)
    msk[:, 0] = np.tile(np.r_[np.ones(16), np.zeros(16)], 4)
    msk[:, 1] = 1.0 - msk[:, 0]
    return w2, w1s, idb, msk


def host_prep_x(xc):
    """xc: [64, 1152, 8] fp32 -> (x3, xdi)."""
    xb = xc.astype(bf16)
    x3 = np.ascontiguousarray(xb.reshape(BL, NQ, 128, d).transpose(2, 1, 3, 0))
    xd = np.ascontiguousarray(xb.transpose(0, 2, 1))        # [64, 8, 1152]
    xdi = np.concatenate([xd, xd], axis=0)                  # [128, 8, 1152]
    xdi = np.ascontiguousarray(
        xdi.reshape(128, d, NQ, 128).transpose(0, 2, 1, 3))  # [128, q, d, 128]
    return x3, xdi


_NC_CACHE = {}


def _get_nc():
    if "nc" not in _NC_CACHE:
        _NC_CACHE["nc"] = build_program()
    return _NC_CACHE["nc"]


def kernel(x, W):
    x = np.asarray(x, dtype=np.float32)
    W = np.asarray(W, dtype=np.float32)
    w2, w1s, idb, msk = host_prep_w(W)
    in_maps = []
    for core in range(8):
        x3, xdi = host_prep_x(x[core * BL:(core + 1) * BL])
        in_maps.append({"x3": x3, "xdi": xdi, "w2": w2, "w1s": w1s,
                        "idb": idb, "msk": msk})
    nc = _get_nc()
    res = run_bass_kernel_spmd(nc, in_maps, list(range(8)))
    out = np.concatenate([res.results[i]["out"] for i in range(8)], axis=0)
    return out.astype(np.float32)


# revision 8
# speedup vs baseline: 1.0199x; 1.0141x over previous
"""DigitCaps dynamic-routing kernel for 8x TRN2 NeuronCores (v3, q-pipelined).

Per core (64 batch), routing never materializes u_hat:
  s0   = 0.1 * x @ W                       (PE, K=(i,d) contraction)
  v    = squash(s) = P(z^2) - 0.5|z| - 1   (polynomial; one act table)
  per q-chunk of 128 input capsules, fully pipelined:
    A(q,p)   = W x_D v          (PE, K=32=(o2,D) per o-pair, out (d,i128))
    prod     = A * x            (ACT evict + DVE mult, some pairs fused on DVE)
    bvalT(q) = sum_d prod       (PE: 8 accumulating identity-matmuls = transpose+tree)
    c(q)     = exp(bvalT)       (ACT, from PSUM)
    y(q)     = c * x/Z          (DVE/Pool split)
    s       += y(q,d,o) @ W     (PE, y as lhsT, per-o N=16 rhs, b-layout PSUM out)
  v_new = squash(s); b2 = b1 + UV(v1) = UV(v0+v1) by linearity.
"""

import os
import numpy as np
import ml_dtypes

import concourse.bass as bass
import concourse.bacc as bacc
import concourse.mybir as mybir
from concourse.tile import TileContext
from concourse.bass_utils import run_bass_kernel_spmd

bf16 = ml_dtypes.bfloat16
F32 = mybir.dt.float32
BF = mybir.dt.bfloat16
AF = mybir.ActivationFunctionType
ALU = mybir.AluOpType
AX = mybir.AxisListType

B, O, I, D, d = 512, 10, 1152, 16, 8
BL = 64          # batch per core
NPAIR = 5        # o-pairs
NQ = 9           # i chunks of 128
NT = 72          # (q, d) tiles

_B = lambda k, dflt: int(os.environ.get(k, dflt))
def _sl(t):
    import concourse.bass as _bass
    return t if isinstance(t, _bass.AP) else t[:]

# squash: v = P(z^2) - 0.5|z| - 1, P(w) ~= cos(sqrt w) - 0.5 sqrt(w) erf(sqrt(w/2))
# fit over |z| <= 1.2, deg 3, max err ~4e-5; -1 folded into c0
_PC = [-2.1933e-05, -0.8983608275706338, 0.10590715814988215,
       -0.008555334079961922]


def _squash(nc, pool, ps_ap, shape, tag, scale, out_dt=F32):
    """v = squash(scale*ps) via poly; ps_ap is a PSUM AP; returns f32 tile.
    Intermediates in bf16 for DVE 2x/4x modes (|err| ~1e-3 << gate)."""
    IDT = BF if _B("SQBF", 1) else F32
    a = pool.tile(shape, IDT, tag=f"{tag}_a")
    w = pool.tile(shape, IDT, tag=f"{tag}_w")
    r = pool.tile(shape, IDT, tag=f"{tag}_r")
    v = pool.tile(shape, out_dt, tag=f"{tag}_v")
    nc.scalar.activation(w[:], ps_ap, AF.Square, scale=scale)
    nc.scalar.activation(a[:], ps_ap, AF.Abs, scale=scale)
    with nc.allow_low_precision(reason="poly intermediates; gate margin 6x"):
        nc.vector.tensor_scalar(r[:], w[:], _PC[3], _PC[2], ALU.mult, ALU.add)
        for k in (1, 0):
            nc.vector.tensor_tensor(r[:], r[:], w[:], ALU.mult)
            nc.vector.tensor_scalar(r[:], r[:], _PC[k], None, ALU.add)
        nc.vector.scalar_tensor_tensor(v[:], a[:], -0.5, r[:], ALU.mult, ALU.add)
    return v


def _body(nc, tc, x3_d, xdi_d, w2_d, w1s_d, idb_d, msk_d, out_d):
    NDQ = _B("NDQ", 5)        # of 45 (q,p) chunks: how many fuse evict+mult on DVE
    if _B("DSP", 0) == 0:
        direct_set = {round(i * 45 / NDQ) + 7 for i in range(NDQ)} if NDQ else set()
    else:
        direct_set = {q * 5 + 3 for q in range(0, 2 * NDQ, 2)} if NDQ else set()
    YPD = _B("YPD", 2)        # y d-slices on Pool (of 8); tail q runs all-DVE
    with (
        tc.tile_pool(name="const", bufs=1) as cpool,
        tc.tile_pool(name="work", bufs=1) as wpool,
        tc.tile_pool(name="small", bufs=3) as spool,
        tc.tile_pool(name="asb", bufs=_B("ASB", 4)) as apool,
        tc.tile_pool(name="ypool", bufs=_B("YB", 3)) as ypool,
        tc.tile_pool(name="psA2", bufs=_B("PSA2", 2), space="PSUM") as psA2_pool,
        tc.tile_pool(name="psBR", bufs=_B("PSBR", 1), space="PSUM") as psBR_pool,
        tc.tile_pool(name="psS", bufs=1, space="PSUM") as psS_pool,
    ):
        # ---- resident loads: x3 whole, w1s in 3 chunk-tiles (fine-grained
        # deps let s0 chase the DMA), then per-q xdi/w2 ----
        x3 = cpool.tile([128, NQ, d, BL], BF)
        w1sc = [cpool.tile([128, 24, 160], BF, name=f"w1s{j}") for j in range(3)]
        xdi = cpool.tile([128, NQ, d, 128], BF)
        w2 = cpool.tile([32, NQ, NPAIR, d * 128], BF)
        idb = cpool.tile([128, 128], BF)
        msk = cpool.tile([128, 2], F32)
        nc.sync.dma_start(idb[:], idb_d.ap())
        nc.sync.dma_start(msk[:], msk_d.ap())
        nc.sync.dma_start(x3[:], x3_d.ap())
        for j in range(3):
            nc.sync.dma_start(w1sc[j][:], w1s_d.ap()[:, 24 * j:24 * (j + 1)])
        for q in range(NQ):
            nc.sync.dma_start(xdi[:, q], xdi_d.ap()[:, q])
            nc.sync.dma_start(w2[:, q], w2_d.ap()[:, q])

        # ---- PE p-state warmup on idb during the input-DMA window ----
        NWARM = _B("NWARM", 0)
        if NWARM:
            warm = psA2_pool.tile([128, 128], F32, tag="warm", bufs=1, name="warm")
            for i in range(NWARM):
                nc.tensor.matmul(warm[:], idb[:], idb[:], start=True, stop=True)
        # ---- phase s0 ----
        ps0 = psS_pool.tile([BL, 160], F32, tag="psS", name="ps0")
        for t in range(NT):
            q, d_ = divmod(t, d)
            nc.tensor.matmul(ps0[:], x3[:, q, d_], w1sc[t // 24][:, t % 24],
                             start=(t == 0), stop=(t == NT - 1))
        VDT = BF if _B("VBF", 1) else F32
        v0b = _squash(nc, wpool, ps0[:], [BL, 160], "sq0", 0.1, VDT)
        vsum_b = wpool.tile([BL, 160], VDT, tag="vsum")
        nc.vector.tensor_copy(vsum_b[:], v0b[:])

        vnew = None
        for it in range(2):
            vin_b = v0b if it == 0 else vsum_b
            # vT via DVE 32x32 stream transposes; blk = masked per-pair lhsT
            vT = wpool.tile([32, NPAIR, BL], VDT, tag="vT", bufs=2, name=f"vT{it}")
            blk = wpool.tile([32, NPAIR, 128], BF, tag="blk", bufs=2, name=f"blk{it}")
            for p in range(NPAIR):
                nc.vector.transpose(vT[:, p, 0:32], vin_b[0:32, 32 * p:32 * p + 32])
                nc.vector.transpose(vT[:, p, 32:64], vin_b[32:64, 32 * p:32 * p + 32])
                nc.vector.tensor_scalar(blk[:, p, 0:64], vT[:, p], msk[0:32, 0:1], None, ALU.mult)
                nc.vector.tensor_scalar(blk[:, p, 64:128], vT[:, p], msk[0:32, 1:2], None, ALU.mult)

            psSa = psS_pool.tile([BL, 160], F32, tag="psS", name=f"psS{it}")
            prev_y = None
            pending_smax = None
            PMQ = _B("PMQ", -1)     # pair index handled by Pool mult (-1: none)
            for q in range(NQ):
                # ---- UV(q): A-matmuls, evict+mult, tree-transposes ----
                if _B("BRSPLIT", 0):
                    pstA = psBR_pool.tile([128, 512], F32, tag="brA", bufs=_B("BRA", 2),
                                          name=f"pstA{it}_{q}")
                    pstB = psBR_pool.tile([128, 128], F32, tag="brB", bufs=_B("BRB", 1),
                                          name=f"pstB{it}_{q}")
                else:
                    pst5 = psBR_pool.tile([128, 640], F32, tag="br", bufs=1,
                                          name=f"pstb{it}_{q}")
                    pstA = pst5[:, 0:512].rearrange("p (x) -> p x")
                    pstB = pst5[:, 512:640].rearrange("p (x) -> p x")
                asbs = []
                def _uv_chunk(p):
                    psA = psA2_pool.tile([128, 1024], F32, tag="psA",
                                         name=f"psA{it}_{q}_{p}")
                    nc.tensor.matmul(psA[:, 0:512], blk[:, p], w2[:, q, p, 0:512],
                                     start=True, stop=True)
                    nc.tensor.matmul(psA[:, 512:1024], blk[:, p], w2[:, q, p, 512:1024],
                                     start=True, stop=True)
                    A_sb = apool.tile([128, d, 128], BF, tag="A_sb")
                    xstripe = xdi[:, q]
                    if (q * NPAIR + p) in direct_set:
                        nc.vector.tensor_tensor(
                            A_sb[:], psA[:].rearrange("p (a b) -> p a b", a=d),
                            xstripe, ALU.mult)
                    else:
                        nc.scalar.copy(A_sb[:].rearrange("p a b -> p (a b)"), psA[:])
                        eng = nc.gpsimd if p == PMQ else nc.vector
                        eng.tensor_tensor(A_sb[:], A_sb[:], xstripe, ALU.mult)
                    asbs.append(A_sb)
                def _tree(p):
                    tgt = _sl(pstB) if p == 4 else _sl(pstA)[:, 128 * p:128 * (p + 1)]
                    for dd in range(d):
                        nc.tensor.matmul(tgt, asbs[p][:, dd], idb[:],
                                         start=(dd == 0 and p in (0, 4)),
                                         stop=(dd == d - 1))
                for p in range(NPAIR):
                    _uv_chunk(p)
                if _B("EXPD", 0) and pending_smax is not None:
                    pending_smax()      # exp(q-1) lands after evicts(q) on ACT
                    pending_smax = None
                if _B("SPOS", 0) == 1 and prev_y is not None:
                    _emit_S(nc, psSa, prev_y, w1sc, q - 1)
                    prev_y = None
                for p in range(NPAIR):
                    _tree(p)
                if _B("SORD", 1) and prev_y is not None:
                    _emit_S(nc, psSa, prev_y, w1sc, q - 1)
                    prev_y = None
                # ---- softmax(q) + y(q) ----
                def _smax(q=q, pstA=pstA, pstB=pstB, pst5=(None if _B("BRSPLIT", 0) else pst5)):
                    nonlocal prev_y
                    cTq = spool.tile([128, O, BL], BF, tag="cT")
                    if _B("BRSPLIT", 0):
                        nc.scalar.activation(cTq[:, 0:8],
                                             _sl(pstA).rearrange("p (o b) -> p o b", o=8), AF.Exp)
                        nc.scalar.activation(cTq[:, 8:10],
                                             _sl(pstB).rearrange("p (o b) -> p o b", o=2), AF.Exp)
                    else:
                        nc.scalar.activation(cTq[:],
                                             pst5[:].rearrange("p (o b) -> p o b", o=O), AF.Exp)
                    Z = spool.tile([128, BL], F32, tag="Z")
                    nc.vector.tensor_reduce(Z[:], cTq[:].rearrange("p o b -> p b o"),
                                            AX.X, ALU.add)
                    rec = spool.tile([128, BL], BF, tag="rec")
                    with nc.allow_low_precision(reason="1/Z feeds bf16 xs anyway"):
                        nc.vector.reciprocal(rec[:], Z[:])
                    xsq = spool.tile([128, d, BL], BF, tag="xsq")
                    nc.vector.tensor_tensor(xsq[:], x3[:, q],
                                            rec[:].unsqueeze(1).broadcast_to((128, d, BL)),
                                            ALU.mult)
                    yq = ypool.tile([128, d, O, BL], BF, tag="y", name=f"y{it}_{q}")
                    ypd = YPD if q < NQ - 1 else 0
                    dsp = d - ypd
                    if q == NQ - 1:
                        # tail q: split so S can start on early d-slices sooner
                        for dh in range(0, d, 2):
                            nc.vector.tensor_tensor(
                                yq[:, dh:dh + 2],
                                cTq[:].unsqueeze(1).broadcast_to((128, 2, O, BL)),
                                xsq[:, dh:dh + 2].unsqueeze(2).broadcast_to((128, 2, O, BL)),
                                ALU.mult)
                    elif dsp:
                        nc.vector.tensor_tensor(
                            yq[:, 0:dsp],
                            cTq[:].unsqueeze(1).broadcast_to((128, dsp, O, BL)),
                            xsq[:, 0:dsp].unsqueeze(2).broadcast_to((128, dsp, O, BL)),
                            ALU.mult)
                    for ddp in range(dsp, d):
                        nc.gpsimd.tensor_tensor(
                            yq[:, ddp:ddp + 1],
                            cTq[:].unsqueeze(1).broadcast_to((128, 1, O, BL)),
                            xsq[:, ddp:ddp + 1].unsqueeze(2).broadcast_to((128, 1, O, BL)),
                            ALU.mult)
                    if prev_y is not None:
                        _emit_S(nc, psSa, prev_y, w1sc, q - 1)
                    prev_y = yq
                if _B("EXPD", 0):
                    pending_smax = _smax
                else:
                    _smax()
            if pending_smax is not None:
                pending_smax()
            _emit_S(nc, psSa, prev_y, w1sc, NQ - 1)

            vnew = _squash(nc, wpool, psSa[:], [BL, 160], "sqi", 1.0,
                           VDT if it == 0 else F32)
            if it == 0:
                nc.vector.tensor_tensor(vsum_b[:], vsum_b[:], vnew[:], ALU.add)

        # ---- output: vnew [64, 160] == out[b, (o, D)] ----
        nc.sync.dma_start(out_d.ap().rearrange("b o D -> b (o D)"), vnew[:])


def _emit_S(nc, psSa, yq, w1sc, q):
    for dd in range(d):
        t = q * d + dd
        for o in range(O):
            nc.tensor.matmul(psSa[:, 16 * o:16 * o + 16], yq[:, dd, o],
                             w1sc[t // 24][:, t % 24, 16 * o:16 * o + 16],
                             start=(t == 0 and o == 0),
                             stop=(t == NT - 1 and o == O - 1))


def build_program():
    nc = bacc.Bacc("TRN2", debug=False, target_bir_lowering=False)
    x3_d = nc.dram_tensor("x3", [128, NQ, d, BL], BF, kind="ExternalInput")
    xdi_d = nc.dram_tensor("xdi", [128, NQ, d, 128], BF, kind="ExternalInput")
    w2_d = nc.dram_tensor("w2", [32, NQ, NPAIR, d * 128], BF, kind="ExternalInput")
    w1s_d = nc.dram_tensor("w1s", [128, NT, 160], BF, kind="ExternalInput")
    idb_d = nc.dram_tensor("idb", [128, 128], BF, kind="ExternalInput")
    msk_d = nc.dram_tensor("msk", [128, 2], F32, kind="ExternalInput")
    out_d = nc.dram_tensor("out", [BL, O, D], F32, kind="ExternalOutput")
    with TileContext(nc) as tc:
        _body(nc, tc, x3_d, xdi_d, w2_d, w1s_d, idb_d, msk_d, out_d)
    nc.compile()
    return nc


def host_prep_w(W):
    """W: [1,10,1152,16,8] fp32 -> (w2, w1s, idb, msk) arrays."""
    Wb = W[0].astype(bf16)
    # w2[(o2,D), q, p, (d, i128)]
    w2 = np.ascontiguousarray(
        Wb.reshape(5, 2, NQ, 128, D, d).transpose(1, 4, 2, 0, 5, 3)
    ).reshape(32, NQ, NPAIR, d * 128)
    w1s = np.ascontiguousarray(
        Wb.reshape(5, 2, NQ, 128, D, d).transpose(3, 2, 5, 0, 1, 4)).reshape(128, NT, 160)
    idb = np.eye(128, dtype=bf16)
    msk = np.zeros((128, 2), np.float32)
    msk[:, 0] = np.tile(np.r_[np.ones(16), np.zeros(16)], 4)
    msk[:, 1] = 1.0 - msk[:, 0]
    return w2, w1s, idb, msk


def host_prep_x(xc):
    """xc: [64, 1152, 8] fp32 -> (x3, xdi)."""
    xb = xc.astype(bf16)
    x3 = np.ascontiguousarray(xb.reshape(BL, NQ, 128, d).transpose(2, 1, 3, 0))
    xd = np.ascontiguousarray(xb.transpose(0, 2, 1))        # [64, 8, 1152]
    xdi = np.concatenate([xd, xd], axis=0)                  # [128, 8, 1152]
    xdi = np.ascontiguousarray(
        xdi.reshape(128, d, NQ, 128).transpose(0, 2, 1, 3))  # [128, q, d, 128]
    return x3, xdi


_NC_CACHE = {}


def _get_nc():
    if "nc" not in _NC_CACHE:
        _NC_CACHE["nc"] = build_program()
    return _NC_CACHE["nc"]


def kernel(x, W):
    x = np.asarray(x, dtype=np.float32)
    W = np.asarray(W, dtype=np.float32)
    w2, w1s, idb, msk = host_prep_w(W)
    in_maps = []
    for core in range(8):
        x3, xdi = host_prep_x(x[core * BL:(core + 1) * BL])
        in_maps.append({"x3": x3, "xdi": xdi, "w2": w2, "w1s": w1s,
                        "idb": idb, "msk": msk})
    nc = _get_nc()
    res = run_bass_kernel_spmd(nc, in_maps, list(range(8)))
    out = np.concatenate([res.results[i]["out"] for i in range(8)], axis=0)
    return out.astype(np.float32)


# revision 9
# speedup vs baseline: 1.0237x; 1.0037x over previous
"""DigitCaps dynamic-routing kernel for 8x TRN2 NeuronCores (v3, q-pipelined).

Per core (64 batch), routing never materializes u_hat:
  s0   = 0.1 * x @ W                       (PE, K=(i,d) contraction)
  v    = squash(s) = P(z^2) - 0.5|z| - 1   (polynomial; one act table)
  per q-chunk of 128 input capsules, fully pipelined:
    A(q,p)   = W x_D v          (PE, K=32=(o2,D) per o-pair, out (d,i128))
    prod     = A * x            (ACT evict + DVE mult, some pairs fused on DVE)
    bvalT(q) = sum_d prod       (PE: 8 accumulating identity-matmuls = transpose+tree)
    c(q)     = exp(bvalT)       (ACT, from PSUM)
    y(q)     = c * x/Z          (DVE/Pool split)
    s       += y(q,d,o) @ W     (PE, y as lhsT, per-o N=16 rhs, b-layout PSUM out)
  v_new = squash(s); b2 = b1 + UV(v1) = UV(v0+v1) by linearity.
"""

import os
import numpy as np
import ml_dtypes

import concourse.bass as bass
import concourse.bacc as bacc
import concourse.mybir as mybir
from concourse.tile import TileContext
from concourse.bass_utils import run_bass_kernel_spmd

bf16 = ml_dtypes.bfloat16
F32 = mybir.dt.float32
BF = mybir.dt.bfloat16
AF = mybir.ActivationFunctionType
ALU = mybir.AluOpType
AX = mybir.AxisListType

B, O, I, D, d = 512, 10, 1152, 16, 8
BL = 64          # batch per core
NPAIR = 5        # o-pairs
NQ = 9           # i chunks of 128
NT = 72          # (q, d) tiles

_B = lambda k, dflt: int(os.environ.get(k, dflt))
def _sl(t):
    import concourse.bass as _bass
    return t if isinstance(t, _bass.AP) else t[:]

# squash: v = P(z^2) - 0.5|z| - 1, P(w) ~= cos(sqrt w) - 0.5 sqrt(w) erf(sqrt(w/2))
# fit over |z| <= 1.2, deg 3, max err ~4e-5; -1 folded into c0
_PC = [-2.1933e-05, -0.8983608275706338, 0.10590715814988215,
       -0.008555334079961922]


def _squash(nc, pool, ps_ap, shape, tag, scale, out_dt=F32):
    """v = squash(scale*ps) via poly; ps_ap is a PSUM AP; returns f32 tile.
    Intermediates in bf16 for DVE 2x/4x modes (|err| ~1e-3 << gate)."""
    IDT = BF if _B("SQBF", 1) else F32
    a = pool.tile(shape, IDT, tag=f"{tag}_a")
    w = pool.tile(shape, IDT, tag=f"{tag}_w")
    r = pool.tile(shape, IDT, tag=f"{tag}_r")
    v = pool.tile(shape, out_dt, tag=f"{tag}_v")
    nc.scalar.activation(w[:], ps_ap, AF.Square, scale=scale)
    nc.scalar.activation(a[:], ps_ap, AF.Abs, scale=scale)
    with nc.allow_low_precision(reason="poly intermediates; gate margin 6x"):
        nc.vector.tensor_scalar(r[:], w[:], _PC[3], _PC[2], ALU.mult, ALU.add)
        nc.vector.tensor_tensor(r[:], r[:], w[:], ALU.mult)
        nc.vector.tensor_scalar(r[:], r[:], _PC[1], None, ALU.add)
        nc.vector.tensor_tensor(r[:], r[:], w[:], ALU.mult)
        # c0 = -2.2e-5 is negligible vs the 5e-3 error budget: folded out
        nc.vector.scalar_tensor_tensor(v[:], a[:], -0.5, r[:], ALU.mult, ALU.add)
    return v


def _body(nc, tc, x3_d, xdi_d, w2_d, w1s_d, idb_d, msk_d, out_d):
    NDQ = _B("NDQ", 5)        # of 45 (q,p) chunks: how many fuse evict+mult on DVE
    if _B("DSP", 0) == 0:
        direct_set = {round(i * 45 / NDQ) + 7 for i in range(NDQ)} if NDQ else set()
    else:
        direct_set = {q * 5 + 3 for q in range(0, 2 * NDQ, 2)} if NDQ else set()
    YPD = _B("YPD", 2)        # y d-slices on Pool (of 8); tail q runs all-DVE
    with (
        tc.tile_pool(name="const", bufs=1) as cpool,
        tc.tile_pool(name="work", bufs=1) as wpool,
        tc.tile_pool(name="small", bufs=3) as spool,
        tc.tile_pool(name="asb", bufs=_B("ASB", 4)) as apool,
        tc.tile_pool(name="ypool", bufs=_B("YB", 3)) as ypool,
        tc.tile_pool(name="psA2", bufs=_B("PSA2", 2), space="PSUM") as psA2_pool,
        tc.tile_pool(name="psBR", bufs=_B("PSBR", 1), space="PSUM") as psBR_pool,
        tc.tile_pool(name="psS", bufs=1, space="PSUM") as psS_pool,
    ):
        # ---- resident loads: x3 whole, w1s in 3 chunk-tiles (fine-grained
        # deps let s0 chase the DMA), then per-q xdi/w2 ----
        x3 = cpool.tile([128, NQ, d, BL], BF)
        w1sc = [cpool.tile([128, 24, 160], BF, name=f"w1s{j}") for j in range(3)]
        xdi = cpool.tile([128, NQ, d, 128], BF)
        w2 = cpool.tile([32, NQ, NPAIR, d * 128], BF)
        idb = cpool.tile([128, 128], BF)
        msk = cpool.tile([128, 2], F32)
        nc.sync.dma_start(idb[:], idb_d.ap())
        nc.sync.dma_start(msk[:], msk_d.ap())
        nc.sync.dma_start(x3[:], x3_d.ap())
        for j in range(3):
            nc.sync.dma_start(w1sc[j][:], w1s_d.ap()[:, 24 * j:24 * (j + 1)])
        for q in range(NQ):
            nc.sync.dma_start(xdi[:, q], xdi_d.ap()[:, q])
            nc.sync.dma_start(w2[:, q], w2_d.ap()[:, q])

        # ---- PE p-state warmup on idb during the input-DMA window ----
        NWARM = _B("NWARM", 0)
        if NWARM:
            warm = psA2_pool.tile([128, 128], F32, tag="warm", bufs=1, name="warm")
            for i in range(NWARM):
                nc.tensor.matmul(warm[:], idb[:], idb[:], start=True, stop=True)
        # ---- phase s0 ----
        ps0 = psS_pool.tile([BL, 160], F32, tag="psS", name="ps0")
        for t in range(NT):
            q, d_ = divmod(t, d)
            nc.tensor.matmul(ps0[:], x3[:, q, d_], w1sc[t // 24][:, t % 24],
                             start=(t == 0), stop=(t == NT - 1))
        VDT = BF if _B("VBF", 1) else F32
        v0b = _squash(nc, wpool, ps0[:], [BL, 160], "sq0", 0.1, VDT)
        vsum_b = wpool.tile([BL, 160], VDT, tag="vsum")
        nc.vector.tensor_copy(vsum_b[:], v0b[:])

        vnew = None
        for it in range(2):
            vin_b = v0b if it == 0 else vsum_b
            # vT via DVE 32x32 stream transposes; blk = masked per-pair lhsT
            vT = wpool.tile([32, NPAIR, BL], VDT, tag="vT", bufs=2, name=f"vT{it}")
            blk = wpool.tile([32, NPAIR, 128], BF, tag="blk", bufs=2, name=f"blk{it}")
            for p in range(NPAIR):
                nc.vector.transpose(vT[:, p, 0:32], vin_b[0:32, 32 * p:32 * p + 32])
                nc.vector.transpose(vT[:, p, 32:64], vin_b[32:64, 32 * p:32 * p + 32])
                nc.vector.tensor_scalar(blk[:, p, 0:64], vT[:, p], msk[0:32, 0:1], None, ALU.mult)
                nc.vector.tensor_scalar(blk[:, p, 64:128], vT[:, p], msk[0:32, 1:2], None, ALU.mult)

            psSa = psS_pool.tile([BL, 160], F32, tag="psS", name=f"psS{it}")
            prev_y = None
            pending_smax = None
            PMQ = _B("PMQ", -1)     # pair index handled by Pool mult (-1: none)
            for q in range(NQ):
                # ---- UV(q): A-matmuls, evict+mult, tree-transposes ----
                if _B("BRSPLIT", 0):
                    pstA = psBR_pool.tile([128, 512], F32, tag="brA", bufs=_B("BRA", 2),
                                          name=f"pstA{it}_{q}")
                    pstB = psBR_pool.tile([128, 128], F32, tag="brB", bufs=_B("BRB", 1),
                                          name=f"pstB{it}_{q}")
                else:
                    pst5 = psBR_pool.tile([128, 640], F32, tag="br", bufs=1,
                                          name=f"pstb{it}_{q}")
                    pstA = pst5[:, 0:512].rearrange("p (x) -> p x")
                    pstB = pst5[:, 512:640].rearrange("p (x) -> p x")
                asbs = []
                def _uv_chunk(p):
                    psA = psA2_pool.tile([128, 1024], F32, tag="psA",
                                         name=f"psA{it}_{q}_{p}")
                    nc.tensor.matmul(psA[:, 0:512], blk[:, p], w2[:, q, p, 0:512],
                                     start=True, stop=True)
                    nc.tensor.matmul(psA[:, 512:1024], blk[:, p], w2[:, q, p, 512:1024],
                                     start=True, stop=True)
                    A_sb = apool.tile([128, d, 128], BF, tag="A_sb")
                    xstripe = xdi[:, q]
                    if (q * NPAIR + p) in direct_set:
                        nc.vector.tensor_tensor(
                            A_sb[:], psA[:].rearrange("p (a b) -> p a b", a=d),
                            xstripe, ALU.mult)
                    else:
                        nc.scalar.copy(A_sb[:].rearrange("p a b -> p (a b)"), psA[:])
                        eng = nc.gpsimd if p == PMQ else nc.vector
                        eng.tensor_tensor(A_sb[:], A_sb[:], xstripe, ALU.mult)
                    asbs.append(A_sb)
                def _tree(p):
                    tgt = _sl(pstB) if p == 4 else _sl(pstA)[:, 128 * p:128 * (p + 1)]
                    for dd in range(d):
                        nc.tensor.matmul(tgt, asbs[p][:, dd], idb[:],
                                         start=(dd == 0 and p in (0, 4)),
                                         stop=(dd == d - 1))
                for p in range(NPAIR):
                    _uv_chunk(p)
                if _B("EXPD", 0) and pending_smax is not None:
                    pending_smax()      # exp(q-1) lands after evicts(q) on ACT
                    pending_smax = None
                if _B("SPOS", 0) == 1 and prev_y is not None:
                    _emit_S(nc, psSa, prev_y, w1sc, q - 1)
                    prev_y = None
                for p in range(NPAIR):
                    _tree(p)
                if _B("SORD", 1) and prev_y is not None:
                    _emit_S(nc, psSa, prev_y, w1sc, q - 1)
                    prev_y = None
                # ---- softmax(q) + y(q) ----
                def _smax(q=q, pstA=pstA, pstB=pstB, pst5=(None if _B("BRSPLIT", 0) else pst5)):
                    nonlocal prev_y
                    cTq = spool.tile([128, O, BL], BF, tag="cT")
                    if _B("BRSPLIT", 0):
                        nc.scalar.activation(cTq[:, 0:8],
                                             _sl(pstA).rearrange("p (o b) -> p o b", o=8), AF.Exp)
                        nc.scalar.activation(cTq[:, 8:10],
                                             _sl(pstB).rearrange("p (o b) -> p o b", o=2), AF.Exp)
                    else:
                        nc.scalar.activation(cTq[:],
                                             pst5[:].rearrange("p (o b) -> p o b", o=O), AF.Exp)
                    Z = spool.tile([128, BL], F32, tag="Z")
                    nc.vector.tensor_reduce(Z[:], cTq[:].rearrange("p o b -> p b o"),
                                            AX.X, ALU.add)
                    rec = spool.tile([128, BL], BF, tag="rec")
                    with nc.allow_low_precision(reason="1/Z feeds bf16 xs anyway"):
                        nc.vector.reciprocal(rec[:], Z[:])
                    xsq = spool.tile([128, d, BL], BF, tag="xsq")
                    nc.vector.tensor_tensor(xsq[:], x3[:, q],
                                            rec[:].unsqueeze(1).broadcast_to((128, d, BL)),
                                            ALU.mult)
                    yq = ypool.tile([128, d, O, BL], BF, tag="y", name=f"y{it}_{q}")
                    ypd = YPD if q < NQ - 1 else 0
                    dsp = d - ypd
                    if q == NQ - 1:
                        # tail q: split so S can start on early d-slices sooner
                        for dh in range(0, d, 2):
                            nc.vector.tensor_tensor(
                                yq[:, dh:dh + 2],
                                cTq[:].unsqueeze(1).broadcast_to((128, 2, O, BL)),
                                xsq[:, dh:dh + 2].unsqueeze(2).broadcast_to((128, 2, O, BL)),
                                ALU.mult)
                    elif dsp:
                        nc.vector.tensor_tensor(
                            yq[:, 0:dsp],
                            cTq[:].unsqueeze(1).broadcast_to((128, dsp, O, BL)),
                            xsq[:, 0:dsp].unsqueeze(2).broadcast_to((128, dsp, O, BL)),
                            ALU.mult)
                    for ddp in range(dsp, d):
                        nc.gpsimd.tensor_tensor(
                            yq[:, ddp:ddp + 1],
                            cTq[:].unsqueeze(1).broadcast_to((128, 1, O, BL)),
                            xsq[:, ddp:ddp + 1].unsqueeze(2).broadcast_to((128, 1, O, BL)),
                            ALU.mult)
                    if prev_y is not None:
                        _emit_S(nc, psSa, prev_y, w1sc, q - 1)
                    prev_y = yq
                if _B("EXPD", 0):
                    pending_smax = _smax
                else:
                    _smax()
            if pending_smax is not None:
                pending_smax()
            _emit_S(nc, psSa, prev_y, w1sc, NQ - 1)

            vnew = _squash(nc, wpool, psSa[:], [BL, 160], "sqi", 1.0,
                           VDT if it == 0 else F32)
            if it == 0:
                nc.vector.tensor_tensor(vsum_b[:], vsum_b[:], vnew[:], ALU.add)

        # ---- output: vnew [64, 160] == out[b, (o, D)] ----
        nc.sync.dma_start(out_d.ap().rearrange("b o D -> b (o D)"), vnew[:])


def _emit_S(nc, psSa, yq, w1sc, q):
    for dd in range(d):
        t = q * d + dd
        for o in range(O):
            nc.tensor.matmul(psSa[:, 16 * o:16 * o + 16], yq[:, dd, o],
                             w1sc[t // 24][:, t % 24, 16 * o:16 * o + 16],
                             start=(t == 0 and o == 0),
                             stop=(t == NT - 1 and o == O - 1))


def build_program():
    nc = bacc.Bacc("TRN2", debug=False, target_bir_lowering=False)
    x3_d = nc.dram_tensor("x3", [128, NQ, d, BL], BF, kind="ExternalInput")
    xdi_d = nc.dram_tensor("xdi", [128, NQ, d, 128], BF, kind="ExternalInput")
    w2_d = nc.dram_tensor("w2", [32, NQ, NPAIR, d * 128], BF, kind="ExternalInput")
    w1s_d = nc.dram_tensor("w1s", [128, NT, 160], BF, kind="ExternalInput")
    idb_d = nc.dram_tensor("idb", [128, 128], BF, kind="ExternalInput")
    msk_d = nc.dram_tensor("msk", [128, 2], F32, kind="ExternalInput")
    out_d = nc.dram_tensor("out", [BL, O, D], F32, kind="ExternalOutput")
    with TileContext(nc) as tc:
        _body(nc, tc, x3_d, xdi_d, w2_d, w1s_d, idb_d, msk_d, out_d)
    nc.compile()
    return nc


def host_prep_w(W):
    """W: [1,10,1152,16,8] fp32 -> (w2, w1s, idb, msk) arrays."""
    Wb = W[0].astype(bf16)
    # w2[(o2,D), q, p, (d, i128)]
    w2 = np.ascontiguousarray(
        Wb.reshape(5, 2, NQ, 128, D, d).transpose(1, 4, 2, 0, 5, 3)
    ).reshape(32, NQ, NPAIR, d * 128)
    w1s = np.ascontiguousarray(
        Wb.reshape(5, 2, NQ, 128, D, d).transpose(3, 2, 5, 0, 1, 4)).reshape(128, NT, 160)
    idb = np.eye(128, dtype=bf16)
    msk = np.zeros((128, 2), np.float32)
    msk[:, 0] = np.tile(np.r_[np.ones(16), np.zeros(16)], 4)
    msk[:, 1] = 1.0 - msk[:, 0]
    return w2, w1s, idb, msk


def host_prep_x(xc):
    """xc: [64, 1152, 8] fp32 -> (x3, xdi)."""
    xb = xc.astype(bf16)
    x3 = np.ascontiguousarray(xb.reshape(BL, NQ, 128, d).transpose(2, 1, 3, 0))
    xd = np.ascontiguousarray(xb.transpose(0, 2, 1))        # [64, 8, 1152]
    xdi = np.concatenate([xd, xd], axis=0)                  # [128, 8, 1152]
    xdi = np.ascontiguousarray(
        xdi.reshape(128, d, NQ, 128).transpose(0, 2, 1, 3))  # [128, q, d, 128]
    return x3, xdi


_NC_CACHE = {}


def _get_nc():
    if "nc" not in _NC_CACHE:
        _NC_CACHE["nc"] = build_program()
    return _NC_CACHE["nc"]


def kernel(x, W):
    x = np.asarray(x, dtype=np.float32)
    W = np.asarray(W, dtype=np.float32)
    w2, w1s, idb, msk = host_prep_w(W)
    in_maps = []
    for core in range(8):
        x3, xdi = host_prep_x(x[core * BL:(core + 1) * BL])
        in_maps.append({"x3": x3, "xdi": xdi, "w2": w2, "w1s": w1s,
                        "idb": idb, "msk": msk})
    nc = _get_nc()
    res = run_bass_kernel_spmd(nc, in_maps, list(range(8)))
    out = np.concatenate([res.results[i]["out"] for i in range(8)], axis=0)
    return out.astype(np.float32)
